# revision 20
# baseline (speedup 1.0000x reference)
"""GATv2Conv message-passing kernel for 8 Trainium2 NeuronCores.

Strategy (receiver-sharded, padded-grid, no collectives):
- Nodes are sorted by in-degree and dealt round-robin to the 8 cores, so each
  core owns ~12.5k receiver nodes with a balanced edge count; consecutive
  128-node tiles have near-uniform degree.  Consecutive tiles are grouped
  into CHUNKS with a shared (padded-up) slot count D_c, so one dma_gather +
  one op-chain covers several tiles (instruction-issue overhead and the
  994ns SWDGE fixed cost amortize; slot padding stays ~2%).
- Each core computes the full sender projection table s_proj = [x|1] @ [Ws;bs]
  on-device into an HBM scratch table (replicated work), and its local
  receiver projection r_proj into SBUF.
- Gather: per chunk, ONE dma_gather fetches all 128*G*D_c sender rows as
  512B quad-slots (4 fp16 rows per descriptor; int16 slot indices reach
  32768*4 = 131072 rows = whole table).  Gathers round-robin over 4 SWDGE
  queues (measured: one queue ~8.3ns/descriptor serially, 4 queues scale
  nearly linearly).  A 3-pass copy_predicated select (1x DVE; no fast mode
  exists for CopyPredicated) picks sender_row%4 from each quad-slot.
- Edge math (measured-rate-driven): mish factor t = 1 - 2/((e^z+1)^2+1)
  via ACT Exp/Square in f32 (keeps (e^z+1)^2 finite) + DVE
  reciprocal_approx_fast (plain nc.vector.reciprocal measured 6.5x slower);
  padded slots get logit bias -60 instead of a post-exp mask multiply;
  softmax without max-subtraction (logits O(5)); alpha normalized FIRST so
  the weighted sum runs in fp16 (TT 2x) with a log-depth in-place tree for
  the k-reduction (single strided reduce measured 2x slower).
- The per-chunk stages are SOFTWARE-PIPELINED across chunks (engines run
  in emission order, so ops are emitted staggered: select/z for chunk i+1,
  ACT exp/square for i+1, mish chain for i, softmax+aggregate for i-1) --
  without this the cross-engine DVE<->ACT handoffs serialize and the
  measured wall time degenerates to the sum of all engine times.
- Output fp16; host converts, inverse-permutes, and zeroes any
  zero-in-degree receivers (their device result is an artifact of padding).
"""

import numpy as np

import concourse.bass as bass
import concourse.bacc as bacc
import concourse.mybir as mybir
import concourse.tile as tile
from concourse.bass_utils import run_bass_kernel_spmd

F32 = mybir.dt.float32
F16 = mybir.dt.float16
I32 = mybir.dt.int32
I16 = mybir.dt.int16
U8 = mybir.dt.uint8

N_NODES = 100000
N_EDGES = 1600000
F = 64
H = 4
HD = 16
NC_CORES = 8
NQ = 4          # SWDGE queues for gather round-robin
GB = 7          # phase-1b groups per SBUF write batch
CAP = 48        # max G*D_c slots per chunk


def _host_prep(x, Ws, bs, Wr, br, aw, ab, senders, receivers):
    """Pure index/layout work: shard nodes+edges, build chunked grid arrays."""
    N = x.shape[0]
    deg = np.bincount(receivers, minlength=N)
    order = np.argsort(deg, kind="stable").astype(np.int64)  # rank -> node
    inv_order = np.empty(N, dtype=np.int64)
    inv_order[order] = np.arange(N)

    rows_per_core = -(-N // NC_CORES)          # 12500
    tiles = -(-rows_per_core // 128)           # 98
    rows_pad = tiles * 128                     # 12544

    # per-tile max degree over the 1024-rank window (common across cores)
    d_pad = np.zeros(tiles * 1024, dtype=np.int64)
    d_pad[: N] = deg[order]
    D_t = d_pad.reshape(tiles, 1024).max(axis=1)
    D_t = np.maximum(D_t, 1)

    # chunk tiles: consecutive tiles share D_c = max(D_t in chunk), G*D_c<=CAP
    chunks = []   # (t0, G, D_c)
    t0 = 0
    while t0 < tiles:
        G = 1
        Dc = D_t[t0]
        while t0 + G < tiles:
            nd = max(Dc, D_t[t0 + G])
            if (G + 1) * nd > CAP:
                break
            G += 1
            Dc = nd
        chunks.append((t0, G, int(Dc)))
        t0 += G
    D_eff = np.empty(tiles, dtype=np.int64)
    for (c0, G, Dc) in chunks:
        D_eff[c0:c0 + G] = Dc
    OFF = np.concatenate([[0], np.cumsum(D_eff)]).astype(np.int64)
    S = int(OFF[-1])

    # edge -> (core, row, k)
    erank = inv_order[receivers]
    e_sort = np.argsort(erank, kind="stable")
    er_sorted = erank[e_sort]
    s_sorted = senders[e_sort]
    grp_start = np.searchsorted(er_sorted, np.arange(N))
    k_all = np.arange(len(er_sorted)) - grp_start[er_sorted]

    core_e = er_sorted % NC_CORES
    row_e = er_sorted // NC_CORES
    t_e = row_e // 128
    p_e = row_e % 128
    col_e = OFF[t_e] + k_all

    # table layout: node n = g*512 + j*128 + p lands at row
    # p*(4*n_grp) + g*4 + j  (partition-major quad-slots).  Quad-slot
    # index q = row//4 = p*n_grp + g (< 25088, int16-reachable), sub-slot j.
    n_grp = -(-N // 512)                       # 196
    n_tab = 128 * 4 * n_grp                    # 100352
    g_n = s_sorted // 512
    rem = s_sorted % 512
    j_n = rem // 128
    p_n = rem % 128
    q_n = (p_n * n_grp + g_n).astype(np.int16)

    # gather indices, wrapped: linear i = k*128 + p lives at SBUF partition
    # i%16, column i//16 (replicated 8x across the 128 partitions)
    i_lin = k_all * 128 + p_e
    col16 = 8 * OFF[t_e] + i_lin // 16
    row16 = i_lin % 16
    idx16 = np.zeros((NC_CORES, 16, 8 * S), dtype=np.int16)
    idx16[core_e, row16, col16] = q_n
    idx16 = np.tile(idx16, (1, 8, 1))          # [8, 128, 8*S]

    # sub-slot select masks (j==1, j==2, j==3); padded slots get logit
    # bias -60 (replaces the post-exp mask multiply)
    subm = np.zeros((NC_CORES, 128, 3 * S), dtype=np.uint8)
    for jj in range(3):
        sel = j_n == (jj + 1)
        subm[core_e[sel], p_e[sel], jj * S + col_e[sel]] = 1
    bias_arr = np.full((NC_CORES, 128, S), -60.0, dtype=np.float16)
    bias_arr[core_e, p_e, col_e] = 0.0

    # x^T padded + ones row, shared across cores
    xT_aug = np.zeros((F + 1, n_grp * 512), dtype=np.float16)
    xT_aug[:F, :N] = x.T
    xT_aug[F, :] = 1.0

    # per-core local x^T (+ones)
    xlT = np.zeros((NC_CORES, F + 1, rows_pad), dtype=np.float16)
    for c in range(NC_CORES):
        rows = order[c::NC_CORES]
        xlT[c, :F, : len(rows)] = x[rows].T
        xlT[c, F, :] = 1.0

    Wsb = np.concatenate([Ws.reshape(F, F), bs.reshape(1, F)], axis=0).astype(np.float16)
    Wrb = np.concatenate([Wr.reshape(F, F), br.reshape(1, F)], axis=0).astype(np.float16)
    aw_rep = np.tile(np.asarray(aw, np.float32).reshape(1, HD), (1, H)).reshape(1, F)
    awb = np.tile(aw_rep, (128, 1)).astype(np.float16)

    meta = dict(
        chunks=chunks,
        OFF=OFF.astype(int).tolist(),
        S=S,
        tiles=tiles,
        rows_pad=rows_pad,
        n_tab=n_tab,
        n_grp=n_grp,
        order=order,
        zero_nodes=np.where(deg == 0)[0],
        ab=float(np.asarray(ab).reshape(-1)[0]),
    )
    ins = dict(xT=xT_aug, xlT=xlT, Wsb=Wsb, Wrb=Wrb, awb=awb,
               idx=idx16, bias=bias_arr, subm=subm)
    return ins, meta


VARIANT = "full"  # full | gather_only | gather_raw | compute_only | phase1_only | empty
NUMTREE = True    # num reduction: log-depth TT tree vs single strided reduce
NSPLIT = 2        # gather instructions per chunk (queue spreading)


def _build_program(meta):
    chunks, OFF, S = meta["chunks"], meta["OFF"], meta["S"]
    tiles, rows_pad, n_tab, n_grp = (
        meta["tiles"], meta["rows_pad"], meta["n_tab"], meta["n_grp"])
    NCH = len(chunks)
    GDmax = max(G * Dc for (_, G, Dc) in chunks)

    nc = bacc.Bacc(num_swdge_queues=NQ)
    xT = nc.declare_dram_parameter("xT", [F + 1, n_grp * 512], F16, isOutput=False)
    xlT = nc.declare_dram_parameter("xlT", [F + 1, rows_pad], F16, isOutput=False)
    Wsb = nc.declare_dram_parameter("Wsb", [F + 1, F], F16, isOutput=False)
    Wrb = nc.declare_dram_parameter("Wrb", [F + 1, F], F16, isOutput=False)
    awb = nc.declare_dram_parameter("awb", [128, F], F16, isOutput=False)
    idxp = nc.declare_dram_parameter("idx", [128, 8 * S], I16, isOutput=False)
    biasp = nc.declare_dram_parameter("bias", [128, S], F16, isOutput=False)
    submp = nc.declare_dram_parameter("subm", [128, 3 * S], U8, isOutput=False)
    outp = nc.declare_dram_parameter("out", [128, tiles * F], F16, isOutput=True)

    AT = mybir.ActivationFunctionType
    ALU = mybir.AluOpType

    with tile.TileContext(nc) as tc:
        with (
            tc.tile_pool(name="dram", bufs=1, space="DRAM") as dpool,
            tc.tile_pool(name="consts", bufs=1) as cpool,
            tc.tile_pool(name="xload", bufs=2) as xpool,
            tc.tile_pool(name="acc", bufs=2) as apool,
            tc.tile_pool(name="pidx", bufs=4) as ipool,
            tc.tile_pool(name="pg", bufs=2) as pg,
            tc.tile_pool(name="pse", bufs=3) as pse,
            tc.tile_pool(name="pz", bufs=2) as pz,
            tc.tile_pool(name="pa", bufs=2) as pa,
            tc.tile_pool(name="pb", bufs=2) as pb,
            tc.tile_pool(name="pw", bufs=2) as pw,
            tc.tile_pool(name="small", bufs=3) as spool,
            tc.tile_pool(name="psum", bufs=3, space="PSUM") as ppool,
        ):
            table = dpool.tile([n_tab, F], F16)
            table_slots = table[:].rearrange("(q s) c -> q (s c)", s=4)
            table_pv = table[:].rearrange("(p r) c -> p (r c)", p=128)

            wsb_sb = cpool.tile([F + 1, F], F16)
            nc.sync.dma_start(out=wsb_sb[:], in_=Wsb[:])
            wrb_sb = cpool.tile([F + 1, F], F16)
            nc.sync.dma_start(out=wrb_sb[:], in_=Wrb[:])
            awh_sb = cpool.tile([128, F], F16)
            nc.sync.dma_start(out=awh_sb[:], in_=awb[:])
            bias_sb = cpool.tile([128, S], F16)
            nc.sync.dma_start(out=bias_sb[:], in_=biasp[:])
            subm_sb = cpool.tile([128, 3 * S], U8)
            nc.sync.dma_start(out=subm_sb[:], in_=submp[:])
            r_sb = cpool.tile([128, tiles * F], F16)

            if VARIANT == "empty":
                ot0 = spool.tile([128, F], F16, tag="ot")
                nc.vector.tensor_copy(ot0[:], awh_sb[:])
                for t in range(tiles):
                    nc.sync.dma_start(out=outp[:, t * F:(t + 1) * F], in_=ot0[:])
            # phase 1a: r_proj for local nodes, resident in SBUF
            n1a = tiles if VARIANT not in ("empty", "nophase1") else 0
            for tb in range(0, n1a, 4):
                te = min(tb + 4, tiles)
                xt = xpool.tile([F + 1, 512], F16, tag="xl")
                nc.sync.dma_start(out=xt[:, : (te - tb) * 128],
                                  in_=xlT[:, tb * 128: te * 128])
                ps = ppool.tile([128, 4 * F], F32, tag="psr")
                for ti, t in enumerate(range(tb, te)):
                    nc.tensor.matmul(ps[:, ti * F:(ti + 1) * F],
                                     lhsT=xt[:, ti * 128:(ti + 1) * 128],
                                     rhs=wrb_sb[:], start=True, stop=True)
                nc.scalar.copy(r_sb[:, tb * F: te * F],
                               ps[:, : (te - tb) * F])

            # phase 1b: s_proj table in HBM, GB groups per contiguous write
            n_batch = -(-n_grp // GB)
            for b in range(n_batch if VARIANT not in ("empty", "nophase1") else 0):
                glo = b * GB
                ghi = min((b + 1) * GB, n_grp)
                acc = apool.tile([128, GB * 4 * F], F16, tag="acc")
                for g0 in range(glo, ghi, 4):
                    g1 = min(g0 + 4, ghi)
                    xg = xpool.tile([F + 1, 4 * 512], F16, tag="xg")
                    nc.sync.dma_start(out=xg[:, : (g1 - g0) * 512],
                                      in_=xT[:, g0 * 512: g1 * 512])
                    for gi, g in enumerate(range(g0, g1)):
                        ps = ppool.tile([128, 4 * F], F32, tag="pss")
                        for j in range(4):
                            nc.tensor.matmul(
                                ps[:, j * F:(j + 1) * F],
                                lhsT=xg[:, gi * 512 + j * 128:
                                        gi * 512 + (j + 1) * 128],
                                rhs=wsb_sb[:], start=True, stop=True)
                        nc.scalar.copy(
                            acc[:, (g - glo) * 4 * F:(g - glo + 1) * 4 * F],
                            ps[:])
                nc.sync.dma_start(
                    out=table_pv[:, glo * 4 * F: ghi * 4 * F],
                    in_=acc[:, : (ghi - glo) * 4 * F])

            if VARIANT == "nophase1":
                nc.vector.memset(r_sb[:], 0.0)
            if VARIANT == "phase1_only":
                nc.sync.dma_start(out=outp[:], in_=r_sb[:])

            do_main = VARIANT in ("full", "gather_only", "gather_raw",
                                  "compute_only", "noselect", "nophase1")
            do_gather = VARIANT != "compute_only"
            do_math = VARIANT in ("full", "compute_only", "noselect",
                                  "nophase1")

            # per-chunk live tiles, keyed by chunk index
            T = {k: {} for k in ("idx", "g", "se", "z", "ex", "wse")}

            def emit_load(i):
                t0, G, Dc = chunks[i]
                GD = G * Dc
                it = ipool.tile([128, 8 * GDmax], I16, tag="idx")
                nc.sync.dma_start(out=it[:, : 8 * GD],
                                  in_=idxp[:, 8 * OFF[t0]: 8 * (OFF[t0] + GD)])
                T["idx"][i] = it

            def emit_gather(i):
                t0, G, Dc = chunks[i]
                GD = G * Dc
                ns = min(NSPLIT, GD)
                bnds = [round(GD * s / ns) for s in range(ns + 1)]
                halves = [(bnds[s], bnds[s + 1]) for s in range(ns)
                          if bnds[s + 1] > bnds[s]]
                it = T["idx"].pop(i)
                g = pg.tile([128, GDmax * 4 * F], F16, tag="g")
                gv = g[:, : GD * 4 * F].rearrange("p (k c) -> p k c", c=4 * F)
                for h, (klo, khi) in enumerate(halves):
                    kk = khi - klo
                    nc.gpsimd.dma_gather(
                        out_ap=gv[:, klo:khi, :],
                        in_ap=table_slots,
                        idxs_ap=it[:, 8 * klo: 8 * khi],
                        num_idxs=128 * kk,
                        num_idxs_reg=128 * kk,
                        elem_size=4 * F,
                        single_packet=False,
                        queue_num=(NSPLIT * i + h) % NQ,
                    )
                T["g"][i] = g

            def emit_sel(i):
                t0, G, Dc = chunks[i]
                GD = G * Dc
                off = OFF[t0]
                KC = GD * F
                se = pse.tile([128, GDmax * F], F16, tag="se")
                sev = se[:, :KC].rearrange("p (k c) -> p k c", c=F)
                if do_gather:
                    g = T["g"].pop(i)
                    gv = g[:, : GD * 4 * F].rearrange("p (k c) -> p k c",
                                                     c=4 * F)
                    if VARIANT == "gather_raw":
                        otg = spool.tile([128, G * F], F16, tag="ot")
                        nc.vector.tensor_copy(
                            otg[:].rearrange("p (g c) -> p g c", c=F),
                            gv[:, 0:G, 0:F])
                        nc.sync.dma_start(out=outp[:, t0 * F:(t0 + G) * F],
                                          in_=otg[:])
                        return
                    nc.scalar.copy(sev, gv[:, :, 0:F])
                    if VARIANT != "noselect":
                        for jj in range(3):
                            mb = subm_sb[:, jj * S + off: jj * S + off + GD][
                                :, :, None].to_broadcast([128, GD, F])
                            nc.vector.copy_predicated(
                                sev, mb, gv[:, :, (jj + 1) * F:(jj + 2) * F])
                    if VARIANT == "gather_only":
                        otg = spool.tile([128, G * F], F16, tag="ot")
                        nc.vector.tensor_copy(
                            otg[:].rearrange("p (g c) -> p g c", c=F),
                            sev[:, 0:G, :])
                        nc.sync.dma_start(out=outp[:, t0 * F:(t0 + G) * F],
                                          in_=otg[:])
                        return
                else:
                    nc.vector.tensor_copy(se[:, :F], r_sb[:, t0 * F:(t0 + 1) * F])
                T["se"][i] = se
                # z = se + re (re broadcast per tile over its Dc slots)
                rq_b = r_sb[:, t0 * F:(t0 + G) * F].rearrange(
                    "p (g c) -> p g c", c=F)[:, :, None, :].to_broadcast(
                    [128, G, Dc, F])
                z = pz.tile([128, GDmax * F], F16, tag="z")
                nc.vector.tensor_tensor(
                    out=z[:, :KC].rearrange("p (g k c) -> p g k c", g=G, c=F),
                    in0=se[:, :KC].rearrange("p (g k c) -> p g k c", g=G, c=F),
                    in1=rq_b, op=ALU.add)
                T["z"][i] = z

            def emit_act1(i):
                # mish factor part 1 on ACT: e^z and (e^z+1)^2, in f32
                if not do_math:
                    return
                t0, G, Dc = chunks[i]
                KC = G * Dc * F
                z = T["z"][i]
                et = pa.tile([128, GDmax * F], F32, tag="A")
                nc.scalar.activation(et[:, :KC], z[:, :KC], AT.Exp)
                q = pb.tile([128, GDmax * F], F32, tag="B")
                nc.scalar.activation(q[:, :KC], et[:, :KC], AT.Square, bias=1.0)
                T["z"][i] = (z, q)

            def emit_mish(i):
                if not do_math:
                    return
                t0, G, Dc = chunks[i]
                GD = G * Dc
                off = OFF[t0]
                KC = GD * F
                z, q = T["z"].pop(i)
                den = pa.tile([128, GDmax * F], F32, tag="A")
                nc.vector.tensor_scalar_add(den[:, :KC], in0=q[:, :KC],
                                            scalar1=1.0)
                rcp = pb.tile([128, GDmax * F], F32, tag="B")
                nc.vector.reciprocal_approx_fast(out=rcp[:, :KC],
                                                 in_=den[:, :KC])
                w2 = pw.tile([128, GDmax * F], F16, tag="w2")
                nc.scalar.activation(w2[:, :KC], rcp[:, :KC], AT.Copy,
                                     bias=1.0, scale=-2.0)
                m = pb.tile([128, GDmax * F], F16, tag="B")
                nc.vector.tensor_tensor(out=m[:, :KC], in0=z[:, :KC],
                                        in1=w2[:, :KC], op=ALU.mult)
                aw_b = awh_sb[:][:, None, :].to_broadcast([128, GD, F])
                mw = pa.tile([128, GDmax * F], F16, tag="A")
                nc.vector.tensor_tensor(
                    out=mw[:, :KC].rearrange("p (k c) -> p k c", c=F),
                    in0=m[:, :KC].rearrange("p (k c) -> p k c", c=F),
                    in1=aw_b, op=ALU.mult)
                logits = spool.tile([128, GDmax * H], F16, tag="logits")
                with nc.allow_low_precision(reason="logits O(5), fp16 fine"):
                    nc.vector.tensor_reduce(
                        out=logits[:, : GD * H],
                        in_=mw[:, :KC].rearrange("p (k h d) -> p k h d",
                                                 h=H, d=HD),
                        axis=mybir.AxisListType.X, op=ALU.add)
                # padded slots -> -60; ab cancels in the softmax -- skip it
                bias_b = bias_sb[:, off:off + GD][:, :, None].to_broadcast(
                    [128, GD, H])
                logb = spool.tile([128, GDmax * H], F16, tag="logb")
                nc.vector.tensor_tensor(
                    out=logb[:, : GD * H].rearrange("p (k h) -> p k h", h=H),
                    in0=logits[:, : GD * H].rearrange("p (k h) -> p k h", h=H),
                    in1=bias_b, op=ALU.add)
                T["ex"][i] = logb

            def emit_ex(i):
                if not do_math:
                    return
                t0, G, Dc = chunks[i]
                GD = G * Dc
                logb = T["ex"].pop(i)
                ex = spool.tile([128, GDmax * H], F32, tag="ex")
                nc.scalar.activation(ex[:, : GD * H], logb[:, : GD * H], AT.Exp)
                T["ex"][i] = ex

            def emit_agg(i):
                if not do_math:
                    return
                t0, G, Dc = chunks[i]
                GD = G * Dc
                KC = GD * F
                ex = T["ex"].pop(i)
                se = T["se"].pop(i)
                den_s = spool.tile([128, G * H], F32, tag="den")
                nc.vector.tensor_reduce(
                    out=den_s[:],
                    in_=ex[:, : GD * H].rearrange("p (g k h) -> p g h k",
                                                  g=G, h=H),
                    axis=mybir.AxisListType.X, op=ALU.add)
                rec = spool.tile([128, G * H], F32, tag="rec")
                nc.vector.reciprocal(rec[:], den_s[:])
                alpha16 = spool.tile([128, GDmax * H], F16, tag="alpha16")
                rec_b = rec[:].rearrange("p (g h) -> p g h", h=H)[
                    :, :, None, :].to_broadcast([128, G, Dc, H])
                with nc.allow_low_precision(reason="alpha<=1 fp16"):
                    nc.vector.tensor_tensor(
                        out=alpha16[:, : GD * H].rearrange(
                            "p (g k h) -> p g k h", g=G, h=H),
                        in0=ex[:, : GD * H].rearrange("p (g k h) -> p g k h",
                                                      g=G, h=H),
                        in1=rec_b, op=ALU.mult)
                wse = pw.tile([128, GDmax * F], F16, tag="W")
                a_b = alpha16[:, : GD * H].rearrange(
                    "p (k h) -> p k h", h=H)[:, :, :, None].to_broadcast(
                    [128, GD, H, HD])
                with nc.allow_low_precision(reason="alpha<=1 fp16 sum"):
                    nc.vector.tensor_tensor(
                        out=wse[:, :KC].rearrange("p (k h d) -> p k h d",
                                                  h=H, d=HD),
                        in0=se[:, :KC].rearrange("p (k h d) -> p k h d",
                                                 h=H, d=HD),
                        in1=a_b, op=ALU.mult)
                    if NUMTREE:
                        wv = wse[:, :KC].rearrange("p (g k c) -> p g k c",
                                                   g=G, c=F)
                        n = Dc
                        while n > 1:
                            half = n // 2
                            nc.vector.tensor_tensor(
                                out=wv[:, :, 0:half, :],
                                in0=wv[:, :, 0:half, :],
                                in1=wv[:, :, n - half:n, :], op=ALU.add)
                            n -= half
                        nc.sync.dma_start(out=outp[:, t0 * F:(t0 + G) * F],
                                          in_=wv[:, :, 0, :])
                        return
                    num = spool.tile([128, G * F], F32, tag="num")
                    nc.vector.tensor_reduce(
                        out=num[:].rearrange("p (g c) -> p g c", c=F),
                        in_=wse[:, :KC].rearrange("p (g k c) -> p g c k",
                                                  g=G, c=F),
                        axis=mybir.AxisListType.X, op=ALU.add)
                nog = spool.tile([128, G * F], F16, tag="nog")
                nc.scalar.copy(nog[:], num[:])
                nc.sync.dma_start(out=outp[:, t0 * F:(t0 + G) * F],
                                  in_=nog[:])

            def guard(fn, j):
                if 0 <= j < NCH:
                    fn(j)

            if do_main:
                for j in (0, 1, 2):
                    guard(emit_load, j)
                if do_gather:
                    guard(emit_gather, 0)
                    guard(emit_gather, 1)
                guard(emit_sel, 0)
                guard(emit_act1, 0)
                for i in range(NCH):
                    guard(emit_load, i + 3)
                    if do_gather:
                        guard(emit_gather, i + 2)
                    guard(emit_sel, i + 1)
                    guard(emit_act1, i + 1)
                    emit_mish(i)
                    emit_ex(i)
                    guard(emit_agg, i - 1)
                guard(emit_agg, NCH - 1)

    return nc


def kernel(x, Ws, bs, Wr, br, aw, ab, senders, receivers):
    x = np.asarray(x, np.float32)
    senders = np.asarray(senders, np.int32)
    receivers = np.asarray(receivers, np.int32)
    ins, meta = _host_prep(x, np.asarray(Ws), np.asarray(bs), np.asarray(Wr),
                           np.asarray(br), np.asarray(aw), np.asarray(ab),
                           senders, receivers)
    nc = _build_program(meta)
    if not nc.is_finalized():
        nc.finalize()
    in_maps = []
    for c in range(NC_CORES):
        in_maps.append({
            "xT": ins["xT"],
            "xlT": ins["xlT"][c],
            "Wsb": ins["Wsb"],
            "Wrb": ins["Wrb"],
            "awb": ins["awb"],
            "idx": ins["idx"][c],
            "bias": ins["bias"][c],
            "subm": ins["subm"][c],
        })
    res = run_bass_kernel_spmd(nc, in_maps, core_ids=list(range(NC_CORES)))
    N = x.shape[0]
    order = meta["order"]
    out_full = np.zeros((N, F), dtype=np.float32)
    tiles = meta["tiles"]
    for c in range(NC_CORES):
        oc = res.results[c]["out"].reshape(128, tiles, F).transpose(
            1, 0, 2).reshape(tiles * 128, F)
        rows = order[c::NC_CORES]
        out_full[rows] = oc[: len(rows)].astype(np.float32)
    out_full[meta["zero_nodes"]] = 0.0
    return out_full
